# revision 16
# baseline (speedup 1.0000x reference)
"""Trainium2 Bass kernel for BinarizeConv2d block:
   y = round(2*clip(BN(conv3x3(x, sign(w))), -1, 1))/2
Data-parallel over batch: 2 images per core on 8 NeuronCores.

Conv strategy: x is staged as fp16 (exact enough: rel err ~1e-2 vs the 2e-2
gate) so one K=32 matmul contracts ci. The 128 partitions hold (g, ci) where
g = (image n, row-half h); the PE runs 16 concurrent 32x32 tiles via
tile_position (32g, 32j), col-group j handling one output-row pair. All 9
conv taps are free-dim offsets into a width-padded SBUF chunk (226 cols,
zero seam staged on host); each (g,j) strip accumulates its 9-tap chain into
PSUM bank 4*pg+g. Epilogue: ACT drains PSUM->SBUF fp32, DVE bn_stats ->
(sum,sumsq), AllReduce over cores, per-channel scale/shift via tiny fp32
matmuls + Newton rsqrt, then ACT affine with bias 2b+1536 and fp16 output
(the fp32->fp16 convert does the round-to-nearest-even at integer grid),
DVE clip to [1534,1538] in one 2x-mode op; host subtracts 1536 and halves.
"""
import sys
sys.path.insert(0, "/opt/trn_rl_repo")
import numpy as np
import ml_dtypes
import concourse.bass as bass
import concourse.bacc as bacc
import concourse.tile as tile
from concourse import mybir
from concourse.bass_utils import run_bass_kernel_spmd
import os as _os
if _os.environ.get("LDWOPT", "0") == "1":
    import concourse.bass_utils as _bu
    _orig_run_command = _bu.run_command
    def _patched_run_command(cmd, *a, **kw):
        cmd = ["--enable-ldw-opt=true" if c == "--enable-ldw-opt=false" else c
               for c in cmd]
        return _orig_run_command(cmd, *a, **kw)
    _bu.run_command = _patched_run_command

F32 = mybir.dt.float32
F16 = mybir.dt.float16

N_CORES = 8
NPC = 2           # images per core
C = 32
H = W = 224
WP = 226          # padded width
NCH = 7           # x chunks per core (16 output rows each, + 2 halo rows)
CROWS = 18        # rows per staged chunk
NSS = 14          # 8-row sub-supers per image half
MAGIC16 = 1536.0  # 1.5 * 2**10 -> fp16 round-to-nearest-even trick
EPS = 1e-5
NTOT = float(N_CORES * NPC * H * W)  # elements per channel globally

_cache = {}


def _build_nc(collective=True, loop_n=1, skip=(), mm_order="t_outer",
              mm_ntaps=9, mm_nfree=450, out_i8=True, pool_chunks=2):
    nc = bacc.Bacc("TRN2", target_bir_lowering=False, debug=False,
                   num_devices=N_CORES)
    xs_ext = nc.declare_dram_parameter("xs", [NCH, 128, CROWS, WP], F16,
                                        isOutput=False)
    s_ext = nc.declare_dram_parameter("s", [128, 9, 32], F16, isOutput=False)
    sel1_ext = nc.declare_dram_parameter("sel1", [128, 32], F32, isOutput=False)
    sel2_ext = nc.declare_dram_parameter("sel2", [32, 128], F32, isOutput=False)
    g_ext = nc.declare_dram_parameter("g", [32, 1], F32, isOutput=False)
    b_ext = nc.declare_dram_parameter("b", [32, 1], F32, isOutput=False)
    y_ext = nc.declare_dram_parameter("y", [NPC, C, H, W],
                                      mybir.dt.int8 if out_i8 else F16,
                                      isOutput=True)

    with tile.TileContext(nc) as tc:
        with (
            tc.tile_pool(name="big", bufs=1) as big,
            tc.tile_pool(name="small", bufs=1) as small,
            tc.tile_pool(name="ph2", bufs=2) as ph2,
            tc.tile_pool(name="psum", bufs=1, space="PSUM") as psum,
            tc.tile_pool(name="dram", bufs=1, space="DRAM") as dram,
        ):
            # x chunks: partition p = 32*(2n+h) + ci ; free = (slot18, WP)
            xb = [big.tile([128, CROWS, WP], F16, name=f"xb{i}", tag=f"x{i}")
                  for i in range(2)]
            # y raw conv: partition p = 32*j + co ; free = (g, ss, i, w)
            y_raw = big.tile([128, 4, NSS, 2, W], F32)
            s_sb = small.tile([128, 9, 32], F16)
            stats_buf = small.tile([128, NSS, 4, 6], F32)
            sel1_sb = small.tile([128, 32], F32)
            sel2_sb = small.tile([32, 128], F32)
            g_sb = small.tile([32, 1], F32)
            b_sb = small.tile([32, 1], F32)
            stats_sq = small.tile([128, 2], F32)
            stats_g = small.tile([128, 2], F32)
            msq_scr = small.tile([128, 112], F32)
            red = small.tile([128, 4], F32)
            t32 = small.tile([32, 2], F32)
            fin = small.tile([32, 8], F32)
            sb32 = small.tile([32, 2], F32)
            ab128 = small.tile([128, 2], F32)

            psum_t = psum.tile([128, 8, 512], F32)

            nc.sync.dma_start(out=s_sb[:], in_=s_ext[:])
            nc.sync.dma_start(out=sel1_sb[:], in_=sel1_ext[:])
            nc.sync.dma_start(out=sel2_sb[:], in_=sel2_ext[:])
            nc.sync.dma_start(out=g_sb[:], in_=g_ext[:])
            nc.sync.dma_start(out=b_sb[:], in_=b_ext[:])

            pfull = psum_t[:]
            pstride = pfull.ap[0][0]

            import contextlib
            loop_cm = tc.For_i(0, loop_n, 1) if loop_n > 1 else contextlib.nullcontext()
            with loop_cm:
                _body(nc, tc, locals())
    nc.compile()
    return nc


def _body(nc, tc, env):
    xb = env["xb"]
    y_raw, s_sb = env["y_raw"], env["s_sb"]
    stats_buf, sel1_sb, sel2_sb = env["stats_buf"], env["sel1_sb"], env["sel2_sb"]
    g_sb, b_sb = env["g_sb"], env["b_sb"]
    stats_sq, stats_g, msq_scr, red = (env["stats_sq"], env["stats_g"],
                                       env["msq_scr"], env["red"])
    t32, fin, sb32, ab128 = env["t32"], env["fin"], env["sb32"], env["ab128"]
    psum_t, dram, ph2 = env["psum_t"], env["dram"], env["ph2"]
    y_ext, xs_ext = env["y_ext"], env["xs_ext"]
    pfull, pstride = env["pfull"], env["pstride"]
    collective = env["collective"]
    skip = env["skip"]

    # ---- phase 1: conv + stats per chunk / sub-super ----
    for c in range(NCH):
        x_c = xb[c % 2]
        if "xdma" not in skip:
            nc.sync.dma_start(out=x_c[:], in_=xs_ext[c])
        xv = x_c.rearrange("p r w -> p (r w)")
        for sl in range(2):
            ss = 2 * c + sl
            pg = sl % 2
            bank0 = pg * 4
            if "mm" not in skip:
                order = env.get("mm_order", "t_outer")
                ntaps = env.get("mm_ntaps", 9)
                nfree = env.get("mm_nfree", 450)
                if order == "t_outer":
                    idx = [(t, g, j) for t in range(ntaps)
                           for g in range(4) for j in range(4)]
                else:
                    idx = [(t, g, j) for g in range(4)
                           for j in range(4) for t in range(ntaps)]
                for t, g, j in idx:
                    kh, kw = divmod(t, 3)
                    off = (8 * sl + 2 * j + kh) * WP + kw
                    nc.tensor.matmul(
                        psum_t[32 * j:32 * j + 32, bank0 + g, 0:nfree],
                        s_sb[32 * g:32 * g + 32, t, :],
                        xv[32 * g:32 * g + 32, off:off + nfree],
                        start=(t == 0), stop=(t == ntaps - 1),
                        tile_position=(32 * g, 32 * j))
            # drain the 4 banks of this sub-super to y_raw (skip seam)
            if "epi" not in skip:
                src = bass.AP(
                    tensor=pfull.tensor, offset=pfull.offset + bank0 * 512,
                    ap=[[pstride, 128], [512, 4], [WP, 2], [1, 224]])
                nc.scalar.copy(y_raw[:, :, ss, :, :], src)
            for g in range(4):
                if "stats" in skip:
                    break
                nc.vector.bn_stats(
                    out=stats_buf[:, ss, g, :],
                    in_=y_raw[:, g, ss].rearrange("p i w -> p (i w)"))

    # ---- local stats -> (sum, sumsq) [128, 2] ----
    if "stats" in skip:
        return
    stats_fl = stats_buf.rearrange("p s b (e t) -> p (s b e) t", e=2, t=3)
    means = stats_fl[:, :, 1]
    ctv = stats_fl[:, :, 2]
    nc.vector.tensor_reduce(red[:, 0:1], means, mybir.AxisListType.X,
                            mybir.AluOpType.add)
    nc.vector.tensor_tensor(msq_scr[:], means, means, mybir.AluOpType.mult)
    nc.vector.tensor_reduce(red[:, 1:2], msq_scr[:], mybir.AxisListType.X,
                            mybir.AluOpType.add)
    nc.vector.tensor_reduce(red[:, 2:3], ctv, mybir.AxisListType.X,
                            mybir.AluOpType.add)
    nc.vector.tensor_scalar_mul(stats_sq[:, 0:1], red[:, 0:1], 224.0)
    nc.vector.tensor_scalar_mul(red[:, 3:4], red[:, 1:2], 224.0)
    nc.vector.tensor_tensor(stats_sq[:, 1:2], red[:, 3:4], red[:, 2:3],
                            mybir.AluOpType.add)

    # ---- all-reduce over 8 cores ----
    cc_in = dram.tile([128, 2], F32)
    cc_out = dram.tile([128, 2], F32)
    if collective:
        nc.gpsimd.dma_start(out=cc_in[:], in_=stats_sq[:])
        nc.gpsimd.collective_compute(
            "AllReduce", mybir.AluOpType.add,
            replica_groups=[list(range(N_CORES))],
            ins=[cc_in.opt()], outs=[cc_out.opt()])
        nc.gpsimd.dma_start(out=stats_g[:], in_=cc_out[:])
    else:
        nc.vector.tensor_scalar_mul(stats_g[:], stats_sq[:], float(N_CORES))

    # ---- combine j groups: [128,2] -> [32,2] via PE ----
    nc.tensor.matmul(psum_t[0:32, 0, 0:2], sel1_sb[:], stats_g[:],
                     start=True, stop=True)
    nc.scalar.copy(t32[:], psum_t[0:32, 0, 0:2])

    # ---- finalize per-channel scale/shift on partitions 0..31 ----
    mean = fin[:, 0:1]
    msqm = fin[:, 1:2]
    v = fin[:, 2:3]
    rec = fin[:, 3:4]
    a_ = fin[:, 4:5]
    bq = fin[:, 5:6]
    cq = fin[:, 6:7]
    sc = fin[:, 7:8]
    inv_n = float(np.float32(1.0) / np.float32(NTOT))
    nc.vector.tensor_scalar_mul(fin[:, 0:2], t32[:, 0:2], inv_n)
    nc.vector.tensor_tensor(a_, mean, mean, mybir.AluOpType.mult)
    # v = (msqm + EPS) - mean^2
    nc.vector.scalar_tensor_tensor(v, msqm, EPS, a_, mybir.AluOpType.add,
                                   mybir.AluOpType.subtract)
    nc.scalar.activation(rec, v, mybir.ActivationFunctionType.Sqrt)
    nc.vector.reciprocal(rec, rec)
    # one Newton polish: rec *= 1.5 - 0.5*v*rec^2
    nc.vector.tensor_tensor(a_, rec, rec, mybir.AluOpType.mult)
    nc.vector.tensor_tensor(bq, v, a_, mybir.AluOpType.mult)
    nc.vector.tensor_scalar(cq, bq, -0.5, 1.5, mybir.AluOpType.mult,
                            mybir.AluOpType.add)
    nc.vector.tensor_tensor(rec, rec, cq, mybir.AluOpType.mult)
    nc.vector.tensor_tensor(sc, g_sb[:], rec, mybir.AluOpType.mult)
    nc.vector.tensor_scalar_mul(sb32[:, 0:1], sc, 2.0)
    nc.vector.tensor_tensor(a_, mean, sc, mybir.AluOpType.mult)
    nc.vector.tensor_tensor(bq, b_sb[:], a_, mybir.AluOpType.subtract)
    bias_off = 0.0 if env.get("out_i8", True) else MAGIC16
    nc.vector.tensor_scalar(sb32[:, 1:2], bq, 2.0, bias_off,
                            mybir.AluOpType.mult, mybir.AluOpType.add)

    # broadcast [32,2] -> [128,2]
    nc.tensor.matmul(psum_t[:, 1, 0:2], sel2_sb[:], sb32[:],
                     start=True, stop=True)
    nc.scalar.copy(ab128[:], psum_t[:, 1, 0:2])

    # ---- phase 2: normalize + quantize + writeback ----
    out_i8 = env.get("out_i8", True)
    pool_chunks = env.get("pool_chunks", 2)
    odt = mybir.dt.int8 if out_i8 else F16
    lo, hi = (2.0, -2.0) if out_i8 else (MAGIC16 + 2.0, MAGIC16 - 2.0)
    yap = y_ext.ap()
    # rows = 112*h + 8*ss + 2*j + i, ss in [0, NSS)
    ci_ = 0
    for sh in range(2):
        for g in range(4):
            n, h = divmod(g, 2)
            if "ph2" in skip:
                break
            s0 = sh * (NSS // 2)
            zin = y_raw[:, g, s0:s0 + 7]
            u = ph2.tile([128, 7, 2, W], F32 if out_i8 else F16, tag="u")
            nc.scalar.activation(u[:], zin,
                                 mybir.ActivationFunctionType.Identity,
                                 bias=ab128[:, 1:2], scale=ab128[:, 0:1])
            o = ph2.tile([128, 7, 2, W], odt, tag="o")
            eng = nc.gpsimd if ci_ < pool_chunks else nc.vector
            eng.tensor_scalar(o[:], u[:], lo, hi,
                              mybir.AluOpType.min, mybir.AluOpType.max)
            ci_ += 1
            for j in range(4):
                dst = bass.AP(
                    tensor=yap.tensor,
                    offset=(yap.offset + n * (C * H * W)
                            + (112 * h + 8 * s0 + 2 * j) * W),
                    ap=[[H * W, 32], [8 * W, 7], [1, 2 * W]])
                nc.sync.dma_start(out=dst, in_=o[32 * j:32 * j + 32])


def _get_nc(**kw):
    key = tuple(sorted((k, tuple(v) if isinstance(v, (list, tuple, set)) else v)
                       for k, v in kw.items()))
    if key not in _cache:
        _cache[key] = _build_nc(**kw)
    return _cache[key]


def _host_consts(weight):
    w_bin = np.where(np.asarray(weight, dtype=np.float32) >= 0, 1.0, -1.0)
    # S[32g + ci, t, co] = w_bin[co, ci, kh, kw], t = kh*3+kw, any g
    wt = np.transpose(w_bin.reshape(C, C, 9), (1, 2, 0))  # [ci, t, co]
    s_np = np.tile(wt.astype(np.float16), (4, 1, 1))
    p = np.arange(128)
    sel1 = (p[:, None] % 32 == np.arange(32)[None, :]).astype(np.float32)
    sel2 = (np.arange(32)[:, None] == p[None, :] % 32).astype(np.float32)
    return s_np, sel1, sel2


def _stage_x(xpad_core):
    # xpad_core: [2, 32, 226, 226] fp16, rows/cols 1..224 hold the image.
    # out[c, 32*(2n+h)+ci, slot, w] = xpad[n, ci, 112h + 16c + slot, w]
    sn, sc_, sr, sw = xpad_core.strides
    v = np.lib.stride_tricks.as_strided(
        xpad_core,
        shape=(NCH, NPC, 2, C, CROWS, WP),
        strides=(16 * sr, sn, 112 * sr, sc_, sr, sw))
    return np.ascontiguousarray(v).reshape(NCH, 128, CROWS, WP)


def make_in_maps(x, weight, gamma, beta):
    x = np.asarray(x, dtype=np.float32)
    xpad = np.zeros((N_CORES * NPC, C, WP, WP), dtype=np.float16)
    xpad[:, :, 1:225, 1:225] = x.astype(np.float16)
    s_np, sel1, sel2 = _host_consts(weight)
    g = np.asarray(gamma, dtype=np.float32).reshape(32, 1)
    b = np.asarray(beta, dtype=np.float32).reshape(32, 1)
    in_maps = []
    for c in range(N_CORES):
        in_maps.append({"xs": _stage_x(xpad[c * NPC:(c + 1) * NPC]),
                        "s": s_np, "sel1": sel1, "sel2": sel2,
                        "g": g, "b": b})
    return in_maps


def kernel(x, weight, gamma, beta):
    nc = _get_nc()
    in_maps = make_in_maps(x, weight, gamma, beta)
    res = run_bass_kernel_spmd(nc, in_maps, list(range(N_CORES)))
    out = np.concatenate([res.results[c]["y"] for c in range(N_CORES)], axis=0)
    out = out.astype(np.float32)
    if out.dtype != np.int8 and res.results[0]["y"].dtype == np.float16:
        out -= MAGIC16
    return out * 0.5


# revision 27
# speedup vs baseline: 1.0079x; 1.0079x over previous
"""Trainium2 Bass kernel for BinarizeConv2d block:
   y = round(2*clip(BN(conv3x3(x, sign(w))), -1, 1))/2
Data-parallel over batch: 2 images per core on 8 NeuronCores.

Conv strategy: x is staged as fp16 (exact enough: rel err ~1e-2 vs the 2e-2
gate) so one K=32 matmul contracts ci. The 128 partitions hold (g, ci) where
g = (image n, row-half h); the PE runs 16 concurrent 32x32 tiles via
tile_position (32g, 32j), col-group j handling one output-row pair. All 9
conv taps are free-dim offsets into a width-padded SBUF chunk (226 cols,
zero seam staged on host); each (g,j) strip accumulates its 9-tap chain into
PSUM bank 4*pg+g. Epilogue: ACT drains PSUM->SBUF fp32, DVE bn_stats ->
(sum,sumsq), AllReduce over cores, per-channel scale/shift via tiny fp32
matmuls + Newton rsqrt, then ACT affine with bias 2b+1536 and fp16 output
(the fp32->fp16 convert does the round-to-nearest-even at integer grid),
DVE clip to [1534,1538] in one 2x-mode op; host subtracts 1536 and halves.
"""
import sys
sys.path.insert(0, "/opt/trn_rl_repo")
import numpy as np
import ml_dtypes
import concourse.bass as bass
import concourse.bacc as bacc
import concourse.tile as tile
from concourse import mybir
from concourse.bass_utils import run_bass_kernel_spmd
import os as _os
if _os.environ.get("LDWOPT", "0") == "1":
    import concourse.bass_utils as _bu
    _orig_run_command = _bu.run_command
    def _patched_run_command(cmd, *a, **kw):
        cmd = ["--enable-ldw-opt=true" if c == "--enable-ldw-opt=false" else c
               for c in cmd]
        return _orig_run_command(cmd, *a, **kw)
    _bu.run_command = _patched_run_command

F32 = mybir.dt.float32
F16 = mybir.dt.float16

N_CORES = 8
NPC = 2           # images per core
C = 32
H = W = 224
WP = 226          # padded width
NCH = 7           # x chunks per core (16 output rows each, + 2 halo rows)
CROWS = 18        # rows per staged chunk
NSS = 14          # 8-row sub-supers per image half
MAGIC16 = 1536.0  # 1.5 * 2**10 -> fp16 round-to-nearest-even trick
EPS = 1e-5
NTOT = float(N_CORES * NPC * H * W)  # elements per channel globally

_cache = {}


def _build_nc(collective=True, loop_n=1, skip=(), mm_order="t_outer",
              mm_ntaps=9, mm_nfree=450, out_i8=True, pool_chunks=0,
              xdma_spread=False, odma_eng="sync"):
    nc = bacc.Bacc("TRN2", target_bir_lowering=False, debug=False,
                   num_devices=N_CORES)
    xs_ext = nc.declare_dram_parameter("xs", [128, 114, WP], F16,
                                        isOutput=False)
    s_ext = nc.declare_dram_parameter("s", [128, 9, 32], F16, isOutput=False)
    sel1_ext = nc.declare_dram_parameter("sel1", [128, 32], F32, isOutput=False)
    sel2_ext = nc.declare_dram_parameter("sel2", [32, 128], F32, isOutput=False)
    g_ext = nc.declare_dram_parameter("g", [32, 1], F32, isOutput=False)
    b_ext = nc.declare_dram_parameter("b", [32, 1], F32, isOutput=False)
    y_ext = nc.declare_dram_parameter("y", [NPC, C, H, W],
                                      mybir.dt.int8 if out_i8 else F16,
                                      isOutput=True)

    with tile.TileContext(nc) as tc:
        with (
            tc.tile_pool(name="big", bufs=1) as big,
            tc.tile_pool(name="small", bufs=1) as small,
            tc.tile_pool(name="ph2", bufs=2) as ph2,
            tc.tile_pool(name="psum", bufs=1, space="PSUM") as psum,
            tc.tile_pool(name="dram", bufs=1, space="DRAM") as dram,
        ):
            # x chunks: partition p = 32*(2n+h) + ci ; free = (slot18, WP)
            xb = [big.tile([128, CROWS, WP], F16, name=f"xb{i}", tag=f"x{i}")
                  for i in range(2)]
            # y raw conv: partition p = 32*j + co ; free = (g, ss, i, w)
            y_raw = big.tile([128, 4, NSS, 2, W], F32)
            s_sb = small.tile([128, 9, 32], F16)
            stats_buf = small.tile([128, NSS, 4, 6], F32)
            sel1_sb = small.tile([128, 32], F32)
            sel2_sb = small.tile([32, 128], F32)
            g_sb = small.tile([32, 1], F32)
            b_sb = small.tile([32, 1], F32)
            stats_sq = small.tile([128, 2], F32)
            stats_g = small.tile([128, 2], F32)
            msq_scr = small.tile([128, 112], F32)
            red = small.tile([128, 4], F32)
            t32 = small.tile([32, 2], F32)
            fin = small.tile([32, 8], F32)
            sb32 = small.tile([32, 2], F32)
            ab128 = small.tile([128, 2], F32)

            psum_t = psum.tile([128, 8, 512], F32)

            nc.sync.dma_start(out=s_sb[:], in_=s_ext[:])
            nc.sync.dma_start(out=sel1_sb[:], in_=sel1_ext[:])
            nc.sync.dma_start(out=sel2_sb[:], in_=sel2_ext[:])
            nc.sync.dma_start(out=g_sb[:], in_=g_ext[:])
            nc.sync.dma_start(out=b_sb[:], in_=b_ext[:])

            pfull = psum_t[:]
            pstride = pfull.ap[0][0]

            import contextlib
            loop_cm = tc.For_i(0, loop_n, 1) if loop_n > 1 else contextlib.nullcontext()
            with loop_cm:
                _body(nc, tc, locals())
    nc.compile()
    return nc


def _body(nc, tc, env):
    xb = env["xb"]
    y_raw, s_sb = env["y_raw"], env["s_sb"]
    stats_buf, sel1_sb, sel2_sb = env["stats_buf"], env["sel1_sb"], env["sel2_sb"]
    g_sb, b_sb = env["g_sb"], env["b_sb"]
    stats_sq, stats_g, msq_scr, red = (env["stats_sq"], env["stats_g"],
                                       env["msq_scr"], env["red"])
    t32, fin, sb32, ab128 = env["t32"], env["fin"], env["sb32"], env["ab128"]
    psum_t, dram, ph2 = env["psum_t"], env["dram"], env["ph2"]
    y_ext, xs_ext = env["y_ext"], env["xs_ext"]
    pfull, pstride = env["pfull"], env["pstride"]
    collective = env["collective"]
    skip = env["skip"]

    # ---- phase 1: conv + stats per chunk / sub-super ----
    dma_engs = ([nc.sync, nc.gpsimd, nc.scalar]
                if env.get("xdma_spread", False) else [nc.sync])
    order = env.get("mm_order", "t_outer")
    ntaps = env.get("mm_ntaps", 9)
    nfree = env.get("mm_nfree", 450)

    def mm(t, g, j, sl):
        kh, kw = divmod(t, 3)
        off = (8 * sl + 2 * j + kh) * WP + kw
        xv = xb[mm.c % 2].rearrange("p r w -> p (r w)")
        nc.tensor.matmul(
            psum_t[32 * j:32 * j + 32, (sl % 2) * 4 + g, 0:nfree],
            s_sb[32 * g:32 * g + 32, t, :],
            xv[32 * g:32 * g + 32, off:off + nfree],
            start=(t == 0), stop=(t == ntaps - 1),
            tile_position=(32 * g, 32 * j))

    xsv = xs_ext.ap()
    for c in range(NCH):
        mm.c = c
        x_c = xb[c % 2]
        if "xdma" not in skip:
            src = bass.AP(tensor=xsv.tensor, offset=xsv.offset + 16 * c * WP,
                          ap=[[114 * WP, 128], [1, CROWS * WP]])
            dma_engs[c % len(dma_engs)].dma_start(
                out=x_c.rearrange("p r w -> p (r w)"), in_=src)
        if "mm" not in skip and order == "ldw_share":
            # both sub-supers of the chunk accumulate together so each
            # (t, g, j) weight load serves 2 matmuls
            for t in range(ntaps):
                for g in range(4):
                    for j in range(4):
                        mm(t, g, j, 0)
                        mm(t, g, j, 1)
        for sl in range(2):
            ss = 2 * c + sl
            bank0 = (sl % 2) * 4
            if "mm" not in skip and order != "ldw_share":
                if order == "t_outer":
                    idx = [(t, g, j) for t in range(ntaps)
                           for g in range(4) for j in range(4)]
                else:
                    idx = [(t, g, j) for g in range(4)
                           for j in range(4) for t in range(ntaps)]
                for t, g, j in idx:
                    mm(t, g, j, sl)
            # drain the 4 banks of this sub-super to y_raw (skip seam)
            if "epi" not in skip:
                src = bass.AP(
                    tensor=pfull.tensor, offset=pfull.offset + bank0 * 512,
                    ap=[[pstride, 128], [512, 4], [WP, 2], [1, 224]])
                nc.scalar.copy(y_raw[:, :, ss, :, :], src)
            for g in range(4):
                if "stats" in skip:
                    break
                nc.vector.bn_stats(
                    out=stats_buf[:, ss, g, :],
                    in_=y_raw[:, g, ss].rearrange("p i w -> p (i w)"))

    # ---- local stats -> (sum, sumsq) [128, 2] ----
    if "stats" in skip:
        return
    stats_fl = stats_buf.rearrange("p s b (e t) -> p (s b e) t", e=2, t=3)
    means = stats_fl[:, :, 1]
    ctv = stats_fl[:, :, 2]
    nc.vector.tensor_reduce(red[:, 0:1], means, mybir.AxisListType.X,
                            mybir.AluOpType.add)
    nc.vector.tensor_tensor(msq_scr[:], means, means, mybir.AluOpType.mult)
    nc.vector.tensor_reduce(red[:, 1:2], msq_scr[:], mybir.AxisListType.X,
                            mybir.AluOpType.add)
    nc.vector.tensor_reduce(red[:, 2:3], ctv, mybir.AxisListType.X,
                            mybir.AluOpType.add)
    nc.vector.tensor_scalar_mul(stats_sq[:, 0:1], red[:, 0:1], 224.0)
    nc.vector.tensor_scalar_mul(red[:, 3:4], red[:, 1:2], 224.0)
    nc.vector.tensor_tensor(stats_sq[:, 1:2], red[:, 3:4], red[:, 2:3],
                            mybir.AluOpType.add)

    # ---- all-reduce over 8 cores ----
    cc_in = dram.tile([128, 2], F32)
    cc_out = dram.tile([128, 2], F32)
    if collective:
        nc.gpsimd.dma_start(out=cc_in[:], in_=stats_sq[:])
        nc.gpsimd.collective_compute(
            "AllReduce", mybir.AluOpType.add,
            replica_groups=[list(range(N_CORES))],
            ins=[cc_in.opt()], outs=[cc_out.opt()])
        nc.gpsimd.dma_start(out=stats_g[:], in_=cc_out[:])
    else:
        nc.vector.tensor_scalar_mul(stats_g[:], stats_sq[:], float(N_CORES))

    # ---- combine j groups: [128,2] -> [32,2] via PE ----
    nc.tensor.matmul(psum_t[0:32, 0, 0:2], sel1_sb[:], stats_g[:],
                     start=True, stop=True)
    nc.scalar.copy(t32[:], psum_t[0:32, 0, 0:2])

    # ---- finalize per-channel scale/shift on partitions 0..31 ----
    mean = fin[:, 0:1]
    msqm = fin[:, 1:2]
    v = fin[:, 2:3]
    rec = fin[:, 3:4]
    a_ = fin[:, 4:5]
    bq = fin[:, 5:6]
    cq = fin[:, 6:7]
    sc = fin[:, 7:8]
    inv_n = float(np.float32(1.0) / np.float32(NTOT))
    nc.vector.tensor_scalar_mul(fin[:, 0:2], t32[:, 0:2], inv_n)
    nc.vector.tensor_tensor(a_, mean, mean, mybir.AluOpType.mult)
    # v = (msqm + EPS) - mean^2
    nc.vector.scalar_tensor_tensor(v, msqm, EPS, a_, mybir.AluOpType.add,
                                   mybir.AluOpType.subtract)
    nc.scalar.activation(rec, v, mybir.ActivationFunctionType.Sqrt)
    nc.vector.reciprocal(rec, rec)
    # one Newton polish: rec *= 1.5 - 0.5*v*rec^2
    nc.vector.tensor_tensor(a_, rec, rec, mybir.AluOpType.mult)
    nc.vector.tensor_tensor(bq, v, a_, mybir.AluOpType.mult)
    nc.vector.tensor_scalar(cq, bq, -0.5, 1.5, mybir.AluOpType.mult,
                            mybir.AluOpType.add)
    nc.vector.tensor_tensor(rec, rec, cq, mybir.AluOpType.mult)
    nc.vector.tensor_tensor(sc, g_sb[:], rec, mybir.AluOpType.mult)
    nc.vector.tensor_scalar_mul(sb32[:, 0:1], sc, 2.0)
    nc.vector.tensor_tensor(a_, mean, sc, mybir.AluOpType.mult)
    nc.vector.tensor_tensor(bq, b_sb[:], a_, mybir.AluOpType.subtract)
    bias_off = 0.0 if env.get("out_i8", True) else MAGIC16
    nc.vector.tensor_scalar(sb32[:, 1:2], bq, 2.0, bias_off,
                            mybir.AluOpType.mult, mybir.AluOpType.add)

    # broadcast [32,2] -> [128,2]
    nc.tensor.matmul(psum_t[:, 1, 0:2], sel2_sb[:], sb32[:],
                     start=True, stop=True)
    nc.scalar.copy(ab128[:], psum_t[:, 1, 0:2])

    # ---- phase 2: normalize + quantize + writeback ----
    out_i8 = env.get("out_i8", True)
    pool_chunks = env.get("pool_chunks", 2)
    odt = mybir.dt.int8 if out_i8 else F16
    lo, hi = (2.0, -2.0) if out_i8 else (MAGIC16 + 2.0, MAGIC16 - 2.0)
    yap = y_ext.ap()
    # rows = 112*h + 8*ss + 2*j + i, ss in [0, NSS)
    ci_ = 0
    for sh in range(2):
        for g in range(4):
            n, h = divmod(g, 2)
            if "ph2" in skip:
                break
            s0 = sh * (NSS // 2)
            zin = y_raw[:, g, s0:s0 + 7]
            u = ph2.tile([128, 7, 2, W], F32 if out_i8 else F16, tag="u")
            nc.scalar.activation(u[:], zin,
                                 mybir.ActivationFunctionType.Identity,
                                 bias=ab128[:, 1:2], scale=ab128[:, 0:1])
            o = ph2.tile([128, 7, 2, W], odt, tag="o")
            eng = nc.gpsimd if ci_ < pool_chunks else nc.vector
            if "clip" not in skip:
                eng.tensor_scalar(o[:], u[:], lo, hi,
                                  mybir.AluOpType.min, mybir.AluOpType.max)
            ci_ += 1
            if "odma" in skip:
                continue
            oeng = {"sync": nc.sync, "gpsimd": nc.gpsimd,
                    "scalar": nc.scalar}[env.get("odma_eng", "sync")]
            for j in range(4):
                dst = bass.AP(
                    tensor=yap.tensor,
                    offset=(yap.offset + n * (C * H * W)
                            + (112 * h + 8 * s0 + 2 * j) * W),
                    ap=[[H * W, 32], [8 * W, 7], [1, 2 * W]])
                oeng.dma_start(out=dst, in_=o[32 * j:32 * j + 32])


def _get_nc(**kw):
    key = tuple(sorted((k, tuple(v) if isinstance(v, (list, tuple, set)) else v)
                       for k, v in kw.items()))
    if key not in _cache:
        _cache[key] = _build_nc(**kw)
    return _cache[key]


def _host_consts(weight):
    w_bin = np.where(np.asarray(weight, dtype=np.float32) >= 0, 1.0, -1.0)
    # S[32g + ci, t, co] = w_bin[co, ci, kh, kw], t = kh*3+kw, any g
    wt = np.transpose(w_bin.reshape(C, C, 9), (1, 2, 0))  # [ci, t, co]
    s_np = np.tile(wt.astype(np.float16), (4, 1, 1))
    p = np.arange(128)
    sel1 = (p[:, None] % 32 == np.arange(32)[None, :]).astype(np.float32)
    sel2 = (np.arange(32)[:, None] == p[None, :] % 32).astype(np.float32)
    return s_np, sel1, sel2


def _stage_x(xpad_core):
    # xpad_core: [2, 32, 226, 226] fp16, rows/cols 1..224 hold the image.
    # out[32*(2n+h)+ci, r, w] = xpad[n, ci, 112h + r, w], r in [0, 114)
    sn, sc_, sr, sw = xpad_core.strides
    v = np.lib.stride_tricks.as_strided(
        xpad_core,
        shape=(NPC, 2, C, 114, WP),
        strides=(sn, 112 * sr, sc_, sr, sw))
    return np.ascontiguousarray(v).reshape(128, 114, WP)


def make_in_maps(x, weight, gamma, beta):
    x = np.asarray(x, dtype=np.float32)
    xpad = np.zeros((N_CORES * NPC, C, WP, WP), dtype=np.float16)
    xpad[:, :, 1:225, 1:225] = x.astype(np.float16)
    s_np, sel1, sel2 = _host_consts(weight)
    g = np.asarray(gamma, dtype=np.float32).reshape(32, 1)
    b = np.asarray(beta, dtype=np.float32).reshape(32, 1)
    in_maps = []
    for c in range(N_CORES):
        in_maps.append({"xs": _stage_x(xpad[c * NPC:(c + 1) * NPC]),
                        "s": s_np, "sel1": sel1, "sel2": sel2,
                        "g": g, "b": b})
    return in_maps


def kernel(x, weight, gamma, beta):
    nc = _get_nc()
    in_maps = make_in_maps(x, weight, gamma, beta)
    res = run_bass_kernel_spmd(nc, in_maps, list(range(N_CORES)))
    out = np.concatenate([res.results[c]["y"] for c in range(N_CORES)], axis=0)
    out = out.astype(np.float32)
    if out.dtype != np.int8 and res.results[0]["y"].dtype == np.float16:
        out -= MAGIC16
    return out * 0.5


# revision 39
# speedup vs baseline: 1.1544x; 1.1454x over previous
"""Trainium2 Bass kernel for BinarizeConv2d block:
   y = round(2*clip(BN(conv3x3(x, sign(w))), -1, 1))/2
Data-parallel over batch: 2 images per core on 8 NeuronCores.

Conv strategy: x is staged as fp16 (exact enough: rel err ~1e-2 vs the 2e-2
gate) so one K=32 matmul contracts ci. The 128 partitions hold (g, ci) where
g = (image n, row-half h); the PE runs 16 concurrent 32x32 tiles via
tile_position (32g, 32j), col-group j handling one output-row pair. All 9
conv taps are free-dim offsets into a width-padded SBUF chunk (226 cols,
zero seam staged on host; x rows shared across chunks via a single
[128, 114, 226] staging buffer). Each (g,j) strip accumulates its 9-tap
chain into PSUM bank 4*pg+g. Epilogue: ACT drains PSUM->SBUF fp32, DVE
bn_stats -> (sum,sumsq), AllReduce over cores, per-channel scale/shift via
tiny fp32 matmuls + Newton rsqrt, then ACT affine (fp32), one DVE
tensor_scalar clip to [-2,2] whose int8 output cast does the
round-to-nearest-even; host multiplies the int8 codes by 0.5.
"""
import sys
sys.path.insert(0, "/opt/trn_rl_repo")
import numpy as np
import ml_dtypes
import concourse.bass as bass
import concourse.bacc as bacc
import concourse.tile as tile
from concourse import mybir
from concourse.bass_utils import run_bass_kernel_spmd
import os as _os
if _os.environ.get("LDWOPT", "0") == "1":
    import concourse.bass_utils as _bu
    _orig_run_command = _bu.run_command
    def _patched_run_command(cmd, *a, **kw):
        cmd = ["--enable-ldw-opt=true" if c == "--enable-ldw-opt=false" else c
               for c in cmd]
        return _orig_run_command(cmd, *a, **kw)
    _bu.run_command = _patched_run_command

F32 = mybir.dt.float32
F16 = mybir.dt.float16

N_CORES = 8
NPC = 2           # images per core
C = 32
H = W = 224
WP = 226          # padded width
NCH = 7           # x chunks per core (16 output rows each, + 2 halo rows)
CROWS = 18        # rows per staged chunk
NSS = 14          # 8-row sub-supers per image half
MAGIC16 = 1536.0  # 1.5 * 2**10 -> fp16 round-to-nearest-even trick
EPS = 1e-5
NTOT = float(N_CORES * NPC * H * W)  # elements per channel globally

_cache = {}


def _build_nc(collective=True, loop_n=1, skip=(), mm_order="t_outer",
              mm_ntaps=9, mm_nfree=450, out_i8=True, pool_chunks=0,
              xdma_spread=False, odma_eng="sync", pack4=False):
    nc = bacc.Bacc("TRN2", target_bir_lowering=False, debug=False,
                   num_devices=N_CORES)
    xs_ext = nc.declare_dram_parameter("xs", [128, 114, WP], F16,
                                        isOutput=False)
    s_ext = nc.declare_dram_parameter("s", [128, 9, 32], F16, isOutput=False)
    sel1_ext = nc.declare_dram_parameter("sel1", [128, 32], F32, isOutput=False)
    sel2_ext = nc.declare_dram_parameter("sel2", [32, 128], F32, isOutput=False)
    g_ext = nc.declare_dram_parameter("g", [32, 1], F32, isOutput=False)
    b_ext = nc.declare_dram_parameter("b", [32, 1], F32, isOutput=False)
    ow = W // 2 if pack4 else W
    y_ext = nc.declare_dram_parameter("y", [NPC, C, H, ow],
                                      mybir.dt.int8 if out_i8 else F16,
                                      isOutput=True)

    with tile.TileContext(nc) as tc:
        with (
            tc.tile_pool(name="big", bufs=1) as big,
            tc.tile_pool(name="small", bufs=1) as small,
            tc.tile_pool(name="ph2", bufs=2) as ph2,
            tc.tile_pool(name="psum", bufs=1, space="PSUM") as psum,
            tc.tile_pool(name="dram", bufs=1, space="DRAM") as dram,
        ):
            # x chunks: partition p = 32*(2n+h) + ci ; free = (slot18, WP)
            xb = [big.tile([128, CROWS, WP], F16, name=f"xb{i}", tag=f"x{i}")
                  for i in range(2)]
            # y raw conv: partition p = 32*j + co ; free = (g, ss, i, w)
            y_raw = big.tile([128, 4, NSS, 2, W], F32)
            s_sb = small.tile([128, 9, 32], F16)
            stats_buf = small.tile([128, NSS, 4, 6], F32)
            sel1_sb = small.tile([128, 32], F32)
            sel2_sb = small.tile([32, 128], F32)
            g_sb = small.tile([32, 1], F32)
            b_sb = small.tile([32, 1], F32)
            stats_sq = small.tile([128, 2], F32)
            stats_g = small.tile([128, 2], F32)
            msq_scr = small.tile([128, 112], F32)
            red = small.tile([128, 4], F32)
            t32 = small.tile([32, 2], F32)
            fin = small.tile([32, 8], F32)
            sb32 = small.tile([32, 2], F32)
            ab128 = small.tile([128, 2], F32)

            psum_t = psum.tile([128, 8, 512], F32)

            nc.sync.dma_start(out=s_sb[:], in_=s_ext[:])
            nc.sync.dma_start(out=sel1_sb[:], in_=sel1_ext[:])
            nc.sync.dma_start(out=sel2_sb[:], in_=sel2_ext[:])
            nc.sync.dma_start(out=g_sb[:], in_=g_ext[:])
            nc.sync.dma_start(out=b_sb[:], in_=b_ext[:])

            pfull = psum_t[:]
            pstride = pfull.ap[0][0]

            import contextlib
            loop_cm = tc.For_i(0, loop_n, 1) if loop_n > 1 else contextlib.nullcontext()
            with loop_cm:
                _body(nc, tc, locals())
    nc.compile()
    return nc


def _body(nc, tc, env):
    xb = env["xb"]
    y_raw, s_sb = env["y_raw"], env["s_sb"]
    stats_buf, sel1_sb, sel2_sb = env["stats_buf"], env["sel1_sb"], env["sel2_sb"]
    g_sb, b_sb = env["g_sb"], env["b_sb"]
    stats_sq, stats_g, msq_scr, red = (env["stats_sq"], env["stats_g"],
                                       env["msq_scr"], env["red"])
    t32, fin, sb32, ab128 = env["t32"], env["fin"], env["sb32"], env["ab128"]
    psum_t, dram, ph2 = env["psum_t"], env["dram"], env["ph2"]
    y_ext, xs_ext = env["y_ext"], env["xs_ext"]
    pfull, pstride = env["pfull"], env["pstride"]
    collective = env["collective"]
    skip = env["skip"]

    # ---- phase 1: conv + stats per chunk / sub-super ----
    dma_engs = ([nc.sync, nc.gpsimd, nc.scalar]
                if env.get("xdma_spread", False) else [nc.sync])
    order = env.get("mm_order", "t_outer")
    ntaps = env.get("mm_ntaps", 9)
    nfree = env.get("mm_nfree", 450)

    def mm(t, g, j, sl):
        kh, kw = divmod(t, 3)
        off = (8 * sl + 2 * j + kh) * WP + kw
        xv = xb[mm.c % 2].rearrange("p r w -> p (r w)")
        nc.tensor.matmul(
            psum_t[32 * j:32 * j + 32, (sl % 2) * 4 + g, 0:nfree],
            s_sb[32 * g:32 * g + 32, t, :],
            xv[32 * g:32 * g + 32, off:off + nfree],
            start=(t == 0), stop=(t == ntaps - 1),
            tile_position=(32 * g, 32 * j))

    xsv = xs_ext.ap()
    for c in range(NCH):
        mm.c = c
        x_c = xb[c % 2]
        if "xdma" not in skip:
            src = bass.AP(tensor=xsv.tensor, offset=xsv.offset + 16 * c * WP,
                          ap=[[114 * WP, 128], [1, CROWS * WP]])
            dma_engs[c % len(dma_engs)].dma_start(
                out=x_c.rearrange("p r w -> p (r w)"), in_=src)
        if "mm" not in skip and order == "ldw_share":
            # both sub-supers of the chunk accumulate together so each
            # (t, g, j) weight load serves 2 matmuls
            for t in range(ntaps):
                for g in range(4):
                    for j in range(4):
                        mm(t, g, j, 0)
                        mm(t, g, j, 1)
        for sl in range(2):
            ss = 2 * c + sl
            bank0 = (sl % 2) * 4
            if "mm" not in skip and order != "ldw_share":
                if order == "t_outer":
                    idx = [(t, g, j) for t in range(ntaps)
                           for g in range(4) for j in range(4)]
                else:
                    idx = [(t, g, j) for g in range(4)
                           for j in range(4) for t in range(ntaps)]
                for t, g, j in idx:
                    mm(t, g, j, sl)
            # drain the 4 banks of this sub-super to y_raw (skip seam)
            if "epi" not in skip:
                src = bass.AP(
                    tensor=pfull.tensor, offset=pfull.offset + bank0 * 512,
                    ap=[[pstride, 128], [512, 4], [WP, 2], [1, 224]])
                nc.scalar.copy(y_raw[:, :, ss, :, :], src)
            for g in range(4):
                if "stats" in skip:
                    break
                nc.vector.bn_stats(
                    out=stats_buf[:, ss, g, :],
                    in_=y_raw[:, g, ss].rearrange("p i w -> p (i w)"))

    # ---- local stats -> (sum, sumsq) [128, 2] ----
    if "stats" in skip:
        return
    stats_fl = stats_buf.rearrange("p s b (e t) -> p (s b e) t", e=2, t=3)
    means = stats_fl[:, :, 1]
    ctv = stats_fl[:, :, 2]
    nc.vector.tensor_reduce(red[:, 0:1], means, mybir.AxisListType.X,
                            mybir.AluOpType.add)
    nc.vector.tensor_tensor(msq_scr[:], means, means, mybir.AluOpType.mult)
    nc.vector.tensor_reduce(red[:, 1:2], msq_scr[:], mybir.AxisListType.X,
                            mybir.AluOpType.add)
    nc.vector.tensor_reduce(red[:, 2:3], ctv, mybir.AxisListType.X,
                            mybir.AluOpType.add)
    nc.vector.tensor_scalar_mul(stats_sq[:, 0:1], red[:, 0:1], 224.0)
    nc.vector.tensor_scalar_mul(red[:, 3:4], red[:, 1:2], 224.0)
    nc.vector.tensor_tensor(stats_sq[:, 1:2], red[:, 3:4], red[:, 2:3],
                            mybir.AluOpType.add)

    # ---- all-reduce over 8 cores ----
    cc_in = dram.tile([128, 2], F32)
    cc_out = dram.tile([128, 2], F32)
    if collective:
        nc.gpsimd.dma_start(out=cc_in[:], in_=stats_sq[:])
        nc.gpsimd.collective_compute(
            "AllReduce", mybir.AluOpType.add,
            replica_groups=[list(range(N_CORES))],
            ins=[cc_in.opt()], outs=[cc_out.opt()])
        nc.gpsimd.dma_start(out=stats_g[:], in_=cc_out[:])
    else:
        nc.vector.tensor_scalar_mul(stats_g[:], stats_sq[:], float(N_CORES))

    # ---- combine j groups: [128,2] -> [32,2] via PE ----
    nc.tensor.matmul(psum_t[0:32, 0, 0:2], sel1_sb[:], stats_g[:],
                     start=True, stop=True)
    nc.scalar.copy(t32[:], psum_t[0:32, 0, 0:2])

    # ---- finalize per-channel scale/shift on partitions 0..31 ----
    mean = fin[:, 0:1]
    msqm = fin[:, 1:2]
    v = fin[:, 2:3]
    rec = fin[:, 3:4]
    a_ = fin[:, 4:5]
    bq = fin[:, 5:6]
    cq = fin[:, 6:7]
    sc = fin[:, 7:8]
    inv_n = float(np.float32(1.0) / np.float32(NTOT))
    nc.vector.tensor_scalar_mul(fin[:, 0:2], t32[:, 0:2], inv_n)
    nc.vector.tensor_tensor(a_, mean, mean, mybir.AluOpType.mult)
    # v = (msqm + EPS) - mean^2
    nc.vector.scalar_tensor_tensor(v, msqm, EPS, a_, mybir.AluOpType.add,
                                   mybir.AluOpType.subtract)
    nc.scalar.activation(rec, v, mybir.ActivationFunctionType.Sqrt)
    nc.vector.reciprocal(rec, rec)
    # one Newton polish: rec *= 1.5 - 0.5*v*rec^2
    nc.vector.tensor_tensor(a_, rec, rec, mybir.AluOpType.mult)
    nc.vector.tensor_tensor(bq, v, a_, mybir.AluOpType.mult)
    nc.vector.tensor_scalar(cq, bq, -0.5, 1.5, mybir.AluOpType.mult,
                            mybir.AluOpType.add)
    nc.vector.tensor_tensor(rec, rec, cq, mybir.AluOpType.mult)
    nc.vector.tensor_tensor(sc, g_sb[:], rec, mybir.AluOpType.mult)
    nc.vector.tensor_scalar_mul(sb32[:, 0:1], sc, 2.0)
    nc.vector.tensor_tensor(a_, mean, sc, mybir.AluOpType.mult)
    nc.vector.tensor_tensor(bq, b_sb[:], a_, mybir.AluOpType.subtract)
    if env.get("pack4", False):
        bias_off = 2.0  # shift quant codes to {0..4} for nibble packing
    elif env.get("out_i8", True):
        bias_off = 0.0
    else:
        bias_off = MAGIC16
    nc.vector.tensor_scalar(sb32[:, 1:2], bq, 2.0, bias_off,
                            mybir.AluOpType.mult, mybir.AluOpType.add)

    # broadcast [32,2] -> [128,2]
    nc.tensor.matmul(psum_t[:, 1, 0:2], sel2_sb[:], sb32[:],
                     start=True, stop=True)
    nc.scalar.copy(ab128[:], psum_t[:, 1, 0:2])

    # ---- phase 2: normalize + quantize + writeback ----
    out_i8 = env.get("out_i8", True)
    pack4 = env.get("pack4", False)
    pool_chunks = env.get("pool_chunks", 2)
    odt = mybir.dt.int8 if out_i8 else F16
    if pack4:
        lo, hi = 4.0, 0.0
    elif out_i8:
        lo, hi = 2.0, -2.0
    else:
        lo, hi = MAGIC16 + 2.0, MAGIC16 - 2.0
    ow = W // 2 if pack4 else W
    yap = y_ext.ap()
    # rows = 112*h + 8*ss + 2*j + i, ss in [0, NSS)
    ci_ = 0
    for sh in range(2):
        for g in range(4):
            n, h = divmod(g, 2)
            if "ph2" in skip:
                break
            s0 = sh * (NSS // 2)
            zin = y_raw[:, g, s0:s0 + 7]
            u = ph2.tile([128, 7, 2, W], F32 if out_i8 else F16, tag="u")
            nc.scalar.activation(u[:], zin,
                                 mybir.ActivationFunctionType.Identity,
                                 bias=ab128[:, 1:2], scale=ab128[:, 0:1])
            o = ph2.tile([128, 7, 2, W], F16 if pack4 else odt, tag="o")
            eng = nc.gpsimd if ci_ < pool_chunks else nc.vector
            if "clip" not in skip:
                eng.tensor_scalar(o[:], u[:], lo, hi,
                                  mybir.AluOpType.min, mybir.AluOpType.max)
            if pack4:
                # pack codes {0..4} of cols w and w+112 into one byte:
                # byte = o[w]<<4 | o[w+112]  (contiguous operands, no stride-2)
                o2 = ph2.tile([128, 7, 2, W // 2], mybir.dt.int8, tag="o2")
                nc.vector.scalar_tensor_tensor(
                    o2[:], o[:, :, :, 0:112], 16.0, o[:, :, :, 112:224],
                    mybir.AluOpType.mult, mybir.AluOpType.add)
                osrc = o2
            else:
                osrc = o
            ci_ += 1
            if "odma" in skip:
                continue
            oeng = {"sync": nc.sync, "gpsimd": nc.gpsimd,
                    "scalar": nc.scalar}[env.get("odma_eng", "sync")]
            for j in range(4):
                dst = bass.AP(
                    tensor=yap.tensor,
                    offset=(yap.offset + n * (C * H * ow)
                            + (112 * h + 8 * s0 + 2 * j) * ow),
                    ap=[[H * ow, 32], [8 * ow, 7], [1, 2 * ow]])
                oeng.dma_start(out=dst, in_=osrc[32 * j:32 * j + 32])


def _get_nc(**kw):
    key = tuple(sorted((k, tuple(v) if isinstance(v, (list, tuple, set)) else v)
                       for k, v in kw.items()))
    if key not in _cache:
        _cache[key] = _build_nc(**kw)
    return _cache[key]


def _host_consts(weight):
    w_bin = np.where(np.asarray(weight, dtype=np.float32) >= 0, 1.0, -1.0)
    # S[32g + ci, t, co] = w_bin[co, ci, kh, kw], t = kh*3+kw, any g
    wt = np.transpose(w_bin.reshape(C, C, 9), (1, 2, 0))  # [ci, t, co]
    s_np = np.tile(wt.astype(np.float16), (4, 1, 1))
    p = np.arange(128)
    sel1 = (p[:, None] % 32 == np.arange(32)[None, :]).astype(np.float32)
    sel2 = (np.arange(32)[:, None] == p[None, :] % 32).astype(np.float32)
    return s_np, sel1, sel2


def _stage_x(xpad_core):
    # xpad_core: [2, 32, 226, 226] fp16, rows/cols 1..224 hold the image.
    # out[32*(2n+h)+ci, r, w] = xpad[n, ci, 112h + r, w], r in [0, 114)
    sn, sc_, sr, sw = xpad_core.strides
    v = np.lib.stride_tricks.as_strided(
        xpad_core,
        shape=(NPC, 2, C, 114, WP),
        strides=(sn, 112 * sr, sc_, sr, sw))
    return np.ascontiguousarray(v).reshape(128, 114, WP)


def make_in_maps(x, weight, gamma, beta):
    x = np.asarray(x, dtype=np.float32)
    xpad = np.zeros((N_CORES * NPC, C, WP, WP), dtype=np.float16)
    xpad[:, :, 1:225, 1:225] = x.astype(np.float16)
    s_np, sel1, sel2 = _host_consts(weight)
    g = np.asarray(gamma, dtype=np.float32).reshape(32, 1)
    b = np.asarray(beta, dtype=np.float32).reshape(32, 1)
    in_maps = []
    for c in range(N_CORES):
        in_maps.append({"xs": _stage_x(xpad[c * NPC:(c + 1) * NPC]),
                        "s": s_np, "sel1": sel1, "sel2": sel2,
                        "g": g, "b": b})
    return in_maps


def kernel(x, weight, gamma, beta):
    nc = _get_nc()
    in_maps = make_in_maps(x, weight, gamma, beta)
    res = run_bass_kernel_spmd(nc, in_maps, list(range(N_CORES)))
    out = np.concatenate([res.results[c]["y"] for c in range(N_CORES)], axis=0)
    if out.shape[-1] == W // 2:  # 4-bit packed: byte = o[w]<<4 | o[w+112]
        full = np.empty(out.shape[:-1] + (W,), dtype=np.float32)
        full[..., :112] = (out >> 4) - 2
        full[..., 112:] = (out & 15) - 2
        return full * 0.5
    out = out.astype(np.float32)
    if res.results[0]["y"].dtype == np.float16:
        out -= MAGIC16
    return out * 0.5
